# revision 1
# baseline (speedup 1.0000x reference)
"""GAT layer (nn_GATNode) Trainium2 Bass kernel.

Sharding: 8 cores; core c handles batch b = c//2 and the head pair
hp = c%2 (heads 2*hp, 2*hp+1) over the full 2048x2048 attention matrix.

Math (per batch b, head h):
  e1[i] = h_t[i] @ (Wq[h] @ a1[h]),  e2[j] = h_t[j] @ (Wk[h] @ a2[h])
  exp(lrelu(e1+e2)) = max(exp(e1+e2), exp(0.2*(e1+e2)))
  Dividing softmax column j by exp(0.2*e2[j]) (cancels in softmax):
    Utilde[i,j] = max(f1[i]*r[j], g1[i]),
      f1 = exp(e1), g1 = exp(0.2*e1), r = exp(0.8*e2)
  A[i,j] = adj[i,j] * Utilde[i,j]          (one 4x tensor_scalar + one 2x
                                            tensor_tensor per tile)
  S[j] = sum_i A[i,j]                       (PE ones-matmul, natural layout)
  h'[i,:] = sum_j A[i,j] * Wh[j,:]/S[j]     (PE matmul on xbar-transposed A)
  out = ELU(h')
"""

import os
import numpy as np

B, N, F, T, H, D = 4, 2048, 256, 8, 4, 64
FT = F + T          # 264
FTP = 384           # padded to 3*128 for the xbar transpose
NT = N // 128       # 16 node tiles
FC = 3              # f chunks of 128 (264 -> 384)

_CACHE = {}


def _build_program():
    import concourse.bass as bass
    import concourse.bacc as bacc
    import concourse.mybir as mybir
    from concourse import tile

    F32 = mybir.dt.float32
    F16 = mybir.dt.float16
    I32 = mybir.dt.int32
    AL = mybir.AluOpType
    ACT = mybir.ActivationFunctionType

    nc = bacc.Bacc("TRN2", target_bir_lowering=False, debug=False)

    adj_d = nc.dram_tensor("adj", [N, N], F16, kind="ExternalInput").ap()
    x_d = nc.dram_tensor("x", [N, F], F32, kind="ExternalInput").ap()
    toh_d = nc.dram_tensor("toh", [N, T], F32, kind="ExternalInput").ap()
    vqk_d = nc.dram_tensor("vqk", [FTP, 4], F16, kind="ExternalInput").ap()
    w_d = nc.dram_tensor("w2", [2, F, D], F32, kind="ExternalInput").ap()
    out_d = nc.dram_tensor("out", [N, 2 * D], F32, kind="ExternalOutput").ap()
    s_scr = [nc.dram_tensor(f"s_scr{h}", [1, N], F32, kind="Internal").ap()
             for h in range(2)]

    with tile.TileContext(nc) as tc:
        with (
            tc.tile_pool(name="persist", bufs=1) as pp,
            tc.tile_pool(name="psS", bufs=1, space="PSUM") as ps_S,
        ):
            # ---- persistent SBUF (atT0/atT1 64KB/part each) ----
            atT_all = pp.tile([128, 2 * NT * N], F16, tag="atTall")
            out_sb = pp.tile([128, NT * 128], F32, tag="outsb")  # 8KB/part
            r_bc = [pp.tile([128, N], F16, tag=f"rbc{h}", name=f"rbc{h}")
                    for h in range(2)]
            wh = [pp.tile([128, NT * D], F16, tag=f"wh{h}", name=f"wh{h}")
                  for h in range(2)]
            f_cols = [pp.tile([128, NT], F32, tag=f"fc{h}", name=f"fc{h}")
                      for h in range(2)]
            g_cols = [pp.tile([128, NT], F32, tag=f"gc{h}", name=f"gc{h}")
                      for h in range(2)]
            ones1 = pp.tile([128, 1], F16, tag="ones1")
            ones128 = pp.tile([128, 128], F16, tag="ones128")
            nc.vector.memset(ones1[:], 1.0)
            nc.vector.memset(ones128[:], 1.0)

            # ---- phase A: h_t^T via xbar transpose; e-vectors; Wh ----
            with (
                tc.tile_pool(name="phA", bufs=2) as pa,
                tc.tile_pool(name="phAps", bufs=2, space="PSUM") as pap,
            ):
                # htT in 4 it-major group tiles: block (it%4)*FTP + ft*128
                htT_g = [pa.tile([128, 4 * FTP], F16, tag=f"htTg{g}",
                                 name=f"htTg{g}", bufs=1) for g in range(4)]

                # batched x/toh load + strided casts into padded h_t layout
                xall = pa.tile([128, NT * F], F32, tag="xall", bufs=1)
                tohall = pa.tile([128, NT * T], F32, tag="tohall", bufs=1)
                ht_all = pa.tile([128, NT * FTP], F16, tag="ht_all", bufs=1)
                nc.gpsimd.dma_start(
                    xall[:].rearrange("p (t c) -> p t c", t=NT),
                    x_d[:].rearrange("(t p) c -> p t c", p=128))
                nc.gpsimd.dma_start(
                    tohall[:].rearrange("p (t c) -> p t c", t=NT),
                    toh_d[:].rearrange("(t p) c -> p t c", p=128))
                nc.gpsimd.memset(
                    ht_all[:].rearrange("p (t c) -> p t c", c=FTP)[:, :, FT:], 0.0)
                nc.scalar.copy(
                    ht_all[:].rearrange("p (t c) -> p t c", c=FTP)[:, :, 0:F],
                    xall[:].rearrange("p (t c) -> p t c", c=F))
                nc.vector.tensor_copy(
                    ht_all[:].rearrange("p (t c) -> p t c", c=FTP)[:, :, F:FT],
                    tohall[:].rearrange("p (t c) -> p t c", c=T))
                with tc.high_priority():
                    for g in range(4):
                        # out as the 3D chunk view (chunk index = middle dim),
                        # the same AP form the proven at-transposes use
                        nc.sync.dma_start_transpose(
                            htT_g[g][:].rearrange("p (i c) -> p i c", c=128),
                            ht_all[:, g * 4 * FTP:(g + 1) * 4 * FTP],
                        )

                # V vectors (Wq @ a1, Wk @ a2) precomputed on host
                vst16 = pa.tile([128, FC * 4], F16, tag="vst16")
                nc.gpsimd.dma_start(
                    vst16[:].rearrange("p (c v) -> p c v", c=FC),
                    vqk_d[:].rearrange("(c p) v -> p c v", p=128))
                vst32 = pa.tile([128, FC * 4], F32, tag="vst32")
                nc.vector.tensor_copy(vst32[:], vst16[:])

                # e-columns [node, vec] accumulated over f-chunks
                e_cols = pap.tile([128, NT * 4], F32, tag="ecols", bufs=1)
                for it in range(NT):
                    for ft in range(FC):
                        nc.tensor.matmul(
                            e_cols[:, it * 4:(it + 1) * 4],
                            htT_g[it // 4][:, (it % 4) * FTP + ft * 128:
                                           (it % 4) * FTP + (ft + 1) * 128],
                            vst16[:, ft * 4:(ft + 1) * 4],
                            start=(ft == 0), stop=(ft == FC - 1))
                for hh in range(2):
                    ecol_h = e_cols[:].rearrange("p (t v) -> p t v", v=4)[
                        :, :, 2 * hh:2 * hh + 1]
                    nc.scalar.activation(f_cols[hh][:], ecol_h, ACT.Exp, scale=1.0)
                    nc.scalar.activation(g_cols[hh][:], ecol_h, ACT.Exp, scale=0.2)

                # r_bc per head: e2 broadcast across partitions, exp(0.8*)
                for hh in range(2):
                    for jc in range(4):
                        psr = pap.tile([128, 512], F32, tag="psr", bufs=1)
                        for ft in range(FC):
                            vkbc = pa.tile([128, 128], F16, tag="vkbc")
                            nc.vector.tensor_scalar(
                                vkbc[:], ones128[:],
                                vst32[:, ft * 4 + 2 * hh + 1: ft * 4 + 2 * hh + 2],
                                None, AL.mult)
                            nc.tensor.matmul(
                                psr[:],
                                vkbc[:],
                                htT_g[jc][:].rearrange(
                                    "p (i c) -> p i c", i=4)[
                                    :, :, ft * 128:(ft + 1) * 128],
                                start=(ft == 0), stop=(ft == FC - 1))
                        nc.scalar.activation(
                            r_bc[hh][:, jc * 512:(jc + 1) * 512], psr[:],
                            ACT.Exp, scale=0.8)

                # Wh per head
                for hh in range(2):
                    wb = []
                    for ft in range(2):
                        wstg2 = pa.tile([128, D], F32, tag="wstg2")
                        nc.gpsimd.dma_start(wstg2[:], w_d[hh, ft * 128:(ft + 1) * 128, :])
                        wb16 = pa.tile([128, D], F16, tag=f"wb16_{hh}_{ft}",
                                       name=f"wb16_{hh}_{ft}")
                        nc.vector.tensor_copy(wb16[:], wstg2[:])
                        wb.append(wb16)
                    for it in range(NT):
                        pswh = pap.tile([128, D], F32, tag="pswh")
                        for ft in range(2):
                            nc.tensor.matmul(
                                pswh[:],
                                htT_g[it // 4][:, (it % 4) * FTP + ft * 128:
                                               (it % 4) * FTP + (ft + 1) * 128],
                                wb[ft][:],
                                start=(ft == 0), stop=(ft == 1))
                        nc.vector.tensor_copy(
                            wh[hh][:, it * D:(it + 1) * D], pswh[:])

            # ---- big loop: both heads interleaved per i-tile ----
            # both heads' column sums in one 4-bank psum [2, 2048]
            sS = ps_S.tile([64, N], F32, tag="sS")
            with tc.tile_pool(name="bigloop", bufs=3) as pb:
                for p2 in range(NT // 2):
                    # two i-tiles x two heads in one tile -> one transpose
                    at4 = pb.tile([128, 4 * N], F16, tag="at4", bufs=2)
                    for toff in range(2):
                        it = p2 * 2 + toff
                        adjt = pb.tile([128, N], F16, tag="adjt", bufs=4)
                        nc.gpsimd.dma_start(
                            adjt[:], adj_d[it * 128:(it + 1) * 128, :])
                        for hh in range(2):
                            u = pb.tile([128, N], F16, tag="u", bufs=2)
                            nc.vector.tensor_scalar(
                                u[:], r_bc[hh][:],
                                f_cols[hh][:, it:it + 1],
                                g_cols[hh][:, it:it + 1],
                                AL.mult, AL.max)
                            q = (toff * 2 + hh) * N
                            nc.vector.tensor_tensor(
                                at4[:, q:q + N], u[:], adjt[:], AL.mult)
                            for jc in range(4):
                                nc.tensor.matmul(
                                    sS[32 * hh:32 * hh + 1,
                                       jc * 512:(jc + 1) * 512],
                                    ones1[:],
                                    at4[:, q + jc * 512: q + (jc + 1) * 512],
                                    start=(it == 0), stop=(it == NT - 1))
                    nc.sync.dma_start_transpose(
                        atT_all[:, p2 * 8192:(p2 + 1) * 8192].rearrange(
                            "p (k c) -> p k c", c=128),
                        at4[:])

            # ---- normalize + main matmuls + ELU ----
            with (
                tc.tile_pool(name="post", bufs=1) as po,
                tc.tile_pool(name="psO", bufs=2, space="PSUM") as ps_O,
            ):
                for hh in range(2):
                    s_row = po.tile([1, N], F32, tag="srow", bufs=1)
                    nc.vector.tensor_scalar_add(s_row[:], sS[32 * hh:32 * hh + 1, :], 1e-30)
                    nc.gpsimd.dma_start(s_scr[hh][:], s_row[:])
                    s_sp = po.tile([128, NT], F32, tag="ssp", bufs=2)
                    nc.gpsimd.dma_start(
                        s_sp[:],
                        s_scr[hh][:].rearrange("o (t p) -> (o p) t", p=128))
                    s_rec = po.tile([128, NT], F32, tag="srec", bufs=2)
                    nc.vector.reciprocal(s_rec[:], s_sp[:])
                    whp = po.tile([128, NT * D], F16, tag="whp", bufs=2)
                    for jt in range(NT):
                        nc.vector.tensor_scalar(
                            whp[:, jt * D:(jt + 1) * D],
                            wh[hh][:, jt * D:(jt + 1) * D],
                            s_rec[:, jt:jt + 1], None, AL.mult)

                    for ig in range(2):
                        pso = ps_O.tile([128, 8 * D], F32, tag="pso")
                        for k in range(8):
                            it2 = ig * 8 + k
                            for jt in range(NT):
                                off = ((it2 // 2) * 8192 + (it2 % 2) * 4096
                                       + (hh * NT + jt) * 128)
                                nc.tensor.matmul(
                                    pso[:, k * D:(k + 1) * D],
                                    atT_all[:, off:off + 128],
                                    whp[:, jt * D:(jt + 1) * D],
                                    start=(jt == 0), stop=(jt == NT - 1))
                        # ELU = relu(x) + (exp(min(x,0)) - 1)
                        hsb = po.tile([128, 8 * D], F32, tag="hsb", bufs=2)
                        nc.scalar.copy(hsb[:], pso[:])
                        tmin = po.tile([128, 8 * D], F32, tag="tmin", bufs=2)
                        nc.vector.tensor_scalar_min(tmin[:], hsb[:], 0.0)
                        texp = po.tile([128, 8 * D], F32, tag="texp", bufs=2)
                        nc.scalar.activation(texp[:], tmin[:], ACT.Exp, scale=1.0)
                        trelu = po.tile([128, 8 * D], F32, tag="trelu", bufs=2)
                        nc.vector.tensor_scalar_max(trelu[:], hsb[:], 0.0)
                        nc.vector.scalar_tensor_tensor(
                            out_sb[:].rearrange("p (t c) -> p t c", c=128)[
                                :, ig * 8:(ig + 1) * 8, hh * D:(hh + 1) * D],
                            texp[:], -1.0, trelu[:], AL.add, AL.add)

            # ---- final store ----
            nc.gpsimd.dma_start(
                out_d[:].rearrange("(t p) d -> p t d", p=128),
                out_sb[:].rearrange("p (t d) -> p t d", d=128))

    nc.compile()
    return nc


def _get_program():
    if "nc" not in _CACHE:
        _CACHE["nc"] = _build_program()
    return _CACHE["nc"]


def kernel(x, adj, type_onehot, Wq, Wk, W, a):
    from concourse.bass_utils import run_bass_kernel_spmd

    nc = _get_program()
    x = np.asarray(x, dtype=np.float32)
    adj = np.asarray(adj, dtype=np.int32)
    toh = np.asarray(type_onehot, dtype=np.float32)
    Wq = np.asarray(Wq, dtype=np.float32)
    Wk = np.asarray(Wk, dtype=np.float32)
    W = np.asarray(W, dtype=np.float32)
    a = np.asarray(a, dtype=np.float32)

    a1 = a[:, :D, 0]
    a2v = a[:, D:, 0]
    vq = np.einsum("hfd,hd->hf", Wq, a1)
    vk = np.einsum("hfd,hd->hf", Wk, a2v)
    in_maps = []
    for c in range(8):
        b, hp = c // 2, c % 2
        hs = slice(2 * hp, 2 * hp + 2)
        vqk = np.zeros((FTP, 4), dtype=np.float16)
        vqk[:FT, 0] = vq[2 * hp].astype(np.float16)
        vqk[:FT, 1] = vk[2 * hp].astype(np.float16)
        vqk[:FT, 2] = vq[2 * hp + 1].astype(np.float16)
        vqk[:FT, 3] = vk[2 * hp + 1].astype(np.float16)
        in_maps.append({
            "adj": np.ascontiguousarray(adj[b].astype(np.float16)),
            "x": np.ascontiguousarray(x[b]),
            "toh": np.ascontiguousarray(toh[b]),
            "vqk": vqk,
            "w2": np.ascontiguousarray(W[hs]),
        })

    trace = bool(int(os.environ.get("GAT_TRACE", "0")))
    res = run_bass_kernel_spmd(nc, in_maps, core_ids=list(range(8)), trace=trace)
    _CACHE["last_result"] = res

    out = np.empty((B, N, H * D), dtype=np.float32)
    for c in range(8):
        b, hp = c // 2, c % 2
        out[b, :, 128 * hp:128 * (hp + 1)] = res.results[c]["out"]
    return out



# revision 34
# speedup vs baseline: 1.3731x; 1.3731x over previous
"""GAT layer (nn_GATNode) Trainium2 Bass kernel — transposed-formation design.

Sharding: 8 cores; core c handles batch b = c//2 and the head pair
hp = c%2 (heads 2*hp, 2*hp+1) over the full 2048x2048 attention matrix.

Math (per batch b, head h), with e1[i] = h_t[i]@(Wq a1), e2[j] = h_t[j]@(Wk a2):
  exp(lrelu(e1+e2)) = max(exp(e1+e2), exp(0.2(e1+e2)))
  Softmax is over i (per column j); dividing column j by exp(e2[j]):
    U[i,j]/exp(e2[j]) -> formed TRANSPOSED:
    atT[j,i] = adjT[j,i] * max(F1_bc[i], G1_bc[i]*rq_col[j])
      F1 = exp(e1), G1 = exp(0.2*e1), rq = exp(-0.8*e2)   [broadcast/col tiles]
  S[j] = sum_i atT[j,i]  (fused into a 4x tensor_scalar accum pass)
  h'[i,:] = sum_j atT[j,i] * Wh[j,:]/S[j]   (PE: lhsT = atT chunks)
  out = ELU(h')

The attention matrix is formed directly in [j partitions, i free] layout
(host supplies adj^T), so no N^2 transposes are needed and S completes
per j-tile, letting the main matmuls stream behind the formation.
"""

import os
import numpy as np

B, N, F, T, H, D = 4, 2048, 256, 8, 4, 64
FT = F + T          # 264
FC = 3              # f chunks of 128 (264 -> 384)
FTP = FC * 128      # padded
NT = N // 128       # 16 node tiles

_CACHE = {}

_DEFAULT_CFG = {
    "pool_max": "none",    # Pool cannot run tensor_tensor on real HW
    "act_s": "all",        # which S-passes go to Act
    "t1_bufs": 3, "u_bufs": 2, "atT_bufs": 3, "adjT_bufs": 6,
    "sdump_bufs": 3,
}


def _pool_max(cfg, jt, hh):
    m = cfg["pool_max"]
    if m == "alt":
        return hh == (jt % 2)
    if m == "none":
        return False
    if m == "all":
        return True
    raise ValueError(m)


def _act_s(cfg, jt, hh):
    m = cfg["act_s"]
    if m == "all":
        return True
    if m == "alt":
        return hh == (jt % 2)
    if m == "altj":
        return (jt % 2) == 0
    if m == "none":
        return False
    raise ValueError(m)


def _build_program(cfg=None):
    import concourse.bass as bass
    import concourse.bacc as bacc
    import concourse.mybir as mybir
    from concourse import tile

    F32 = mybir.dt.float32
    F16 = mybir.dt.float16
    AL = mybir.AluOpType
    ACT = mybir.ActivationFunctionType

    cfg = dict(_DEFAULT_CFG, **(cfg or {}))
    nc = bacc.Bacc("TRN2", target_bir_lowering=False, debug=False)

    adjT_d = nc.dram_tensor("adjT", [N, N], F16, kind="ExternalInput").ap()
    htT_d = nc.dram_tensor("htT", [FTP, N], F16, kind="ExternalInput").ap()
    vqk_d = nc.dram_tensor("vqk", [FTP, 4], F16, kind="ExternalInput").ap()
    w_d = nc.dram_tensor("w2", [2, F, D], F16, kind="ExternalInput").ap()
    out_d = nc.dram_tensor("out", [N, 2 * D], F16, kind="ExternalOutput").ap()

    with tile.TileContext(nc) as tc:
        with tc.tile_pool(name="persist", bufs=1) as pp:
            # ---- persistent SBUF ----
            htT_c = [pp.tile([128, N], F16, tag=f"htT{c}", name=f"htT{c}")
                     for c in range(FC)]                  # 12KB/part total
            vst = pp.tile([128, FC, 4], F16, tag="vst")
            wsb = pp.tile([128, 4, D], F16, tag="wsb")            # (hh,c) x D
            ones128 = pp.tile([128, 128], F16, tag="ones128")
            F1bc = [pp.tile([128, N], F16, tag=f"F1bc{h}", name=f"F1bc{h}")
                    for h in range(2)]
            G1bc = [pp.tile([128, N], F16, tag=f"G1bc{h}", name=f"G1bc{h}")
                    for h in range(2)]
            rq_col = [pp.tile([128, NT], F32, tag=f"rq{h}", name=f"rq{h}")
                      for h in range(2)]
            wh = [pp.tile([128, NT * D], F16, tag=f"wh{h}", name=f"wh{h}")
                  for h in range(2)]
            out_sb = pp.tile([128, NT * 128], F16, tag="outsb")   # 4KB/part

            nc.vector.memset(ones128[:], 1.0)

            # ---- phase A: load inputs, e-vectors, broadcasts, Wh ----
            with tc.tile_pool(name="phAps", bufs=1, space="PSUM") as pap:
                nc.sync.dma_start(
                    vst[:], vqk_d[:].rearrange("(c p) v -> p c v", p=128))
                nc.sync.dma_start(
                    htT_c[0][:], htT_d[0:128, :])
                nc.sync.dma_start(
                    wsb[:], w_d[:].rearrange("h (c p) d -> p (h c) d", p=128))
                for c in range(1, FC):
                    nc.sync.dma_start(
                        htT_c[c][:], htT_d[c * 128:(c + 1) * 128, :])

                vst32 = pp.tile([128, FC, 4], F32, tag="vst32")
                nc.vector.tensor_copy(vst32[:], vst[:])

                # PE p-state warmup reading htT chunk 0: starts as soon as the
                # first DMA lands so the ramp is warm when the wide broadcast
                # matmuls issue (idle resets the p-state, so chain off real
                # input data rather than running at t=0)
                warm = pap.tile([128, 128], F32, tag="warm")
                for r in range(14):
                    nc.tensor.matmul(warm[:], htT_c[0][:, 0:128], ones128[:],
                                     start=(r == 0), stop=(r == 13))

                # e columns -> rq_col, chunk-major so they start right
                # behind each htT chunk's DMA (rq_col[0] is loop-critical)
                ec = pap.tile([128, NT * 4], F32, tag="ec")
                for it in range(NT):
                    for c in range(FC):
                        nc.tensor.matmul(
                            ec[:, it * 4:(it + 1) * 4],
                            htT_c[c][:, it * 128:(it + 1) * 128],
                            vst[:, c, :],
                            start=(c == 0), stop=(c == FC - 1))
                nc.scalar.activation(
                    rq_col[0][:],
                    ec[:].rearrange("p (t v) -> p t v", v=4)[:, :, 1:2],
                    ACT.Exp, scale=-0.8)

                # F1/G1 broadcast tiles (loop-critical, h0 first) via
                # replicated-vq ones-matmul + exp, in halves to cut latency
                NH = N // 2
                for hh in range(2):
                    vqrep = pp.tile([128, FC * 128], F16, tag="vqrep",
                                    name="vqrep", bufs=2)
                    for c in range(FC):
                        nc.vector.tensor_scalar(
                            vqrep[:, c * 128:(c + 1) * 128], ones128[:],
                            vst32[:, c, 2 * hh:2 * hh + 1], None, AL.mult)
                    for half in range(2):
                        e1bc = pap.tile([128, NH], F32, tag="e1bc", bufs=2)
                        sl = slice(half * NH, (half + 1) * NH)
                        for q in range(NH // 512):
                            qs = slice(half * NH + q * 512,
                                       half * NH + (q + 1) * 512)
                            for c in range(FC):
                                nc.tensor.matmul(
                                    e1bc[:, q * 512:(q + 1) * 512],
                                    vqrep[:, c * 128:(c + 1) * 128],
                                    htT_c[c][:, qs],
                                    start=(c == 0), stop=(c == FC - 1))
                        nc.scalar.activation(G1bc[hh][:, sl], e1bc[:],
                                             ACT.Exp, scale=0.2)
                        nc.scalar.activation(F1bc[hh][:, sl], e1bc[:],
                                             ACT.Exp, scale=1.0)
                    if hh == 0:
                        nc.scalar.activation(
                            rq_col[1][:],
                            ec[:].rearrange("p (t v) -> p t v", v=4)[
                                :, :, 3:4],
                            ACT.Exp, scale=-0.8)

                # Wh per head: [j,d] = x[j,:] @ W  (htT chunks 0,1 are x^T)
                for hh in range(2):
                    wps = pap.tile([128, NT * D], F32, tag="wps", bufs=1)
                    for jt in range(NT):
                        for c in range(2):
                            nc.tensor.matmul(
                                wps[:, jt * D:(jt + 1) * D],
                                htT_c[c][:, jt * 128:(jt + 1) * 128],
                                wsb[:, 2 * hh + c, :],
                                start=(c == 0), stop=(c == 1))
                    nc.scalar.activation(wh[hh][:], wps[:], ACT.Copy)

            # ---- phase B: j-tile loop; formation + S + streaming matmuls ----
            with (
                tc.tile_pool(name="loop", bufs=3) as pb,
                tc.tile_pool(name="psO", bufs=1, space="PSUM") as ps_o,
            ):

                pso = [ps_o.tile([64, N], F32, tag=f"pso{h}", name=f"pso{h}")
                       for h in range(2)]
                for jt in range(NT):
                    adjT_sb = pb.tile([128, N], F16, tag="adjT", bufs=cfg["adjT_bufs"])
                    nc.sync.dma_start(
                        adjT_sb[:], adjT_d[jt * 128:(jt + 1) * 128, :])
                    for hh in range(2):
                        t1 = pb.tile([128, N], F16, tag=f"t1_{hh}", bufs=cfg["t1_bufs"])
                        nc.vector.tensor_scalar(
                            t1[:], G1bc[hh][:], rq_col[hh][:, jt:jt + 1],
                            None, AL.mult)
                        # max pass: one head per jt on the idle Pool engine
                        on_pool = _pool_max(cfg, jt, hh)
                        eng_max = nc.gpsimd if on_pool else nc.vector
                        utag = f"u{int(on_pool)}_{hh}"
                        u = pb.tile([128, N], F16, tag=utag, name=utag,
                                    bufs=cfg["u_bufs"])
                        eng_max.tensor_tensor(u[:], t1[:], F1bc[hh][:], AL.max)
                        atT = pb.tile([128, N], F16, tag=f"atT_{hh}", bufs=cfg["atT_bufs"])
                        nc.vector.tensor_tensor(atT[:], u[:], adjT_sb[:], AL.mult)
                        # S for these 128 j's: Act-engine copy with accumulate
                        sc = pb.tile([128, 1], F32, tag=f"sc_{hh}", bufs=4)
                        if _act_s(cfg, jt, hh):
                            sdump = pb.tile([128, N], F16, tag="sdump",
                                            bufs=cfg["sdump_bufs"])
                            nc.scalar.activation(
                                sdump[:], atT[:], ACT.Copy, accum_out=sc[:])
                        else:
                            sdump = pb.tile([128, N], F16, tag="sdump",
                                            bufs=cfg["sdump_bufs"])
                            nc.vector.tensor_scalar(
                                sdump[:], atT[:], 1.0, None, AL.mult,
                                accum_out=sc[:])
                        # normalize Wh rows by 1/S
                        se = pb.tile([128, 1], F32, tag=f"se_{hh}", bufs=4)
                        nc.vector.tensor_scalar_add(se[:], sc[:], 1e-30)
                        srec = pb.tile([128, 1], F32, tag=f"srec_{hh}", bufs=4)
                        nc.vector.reciprocal(srec[:], se[:])
                        whp = pb.tile([128, D], F16, tag=f"whp_{hh}", bufs=4)
                        nc.vector.tensor_scalar(
                            whp[:], wh[hh][:, jt * D:(jt + 1) * D],
                            srec[:], None, AL.mult)
                        # h'^T[d, i] += whp^T atT: one psum bank per
                        # 512-chunk so accumulation groups never share a bank
                        for q in range(N // 512):
                            nc.tensor.matmul(
                                pso[hh][:, q * 512:(q + 1) * 512],
                                whp[:],
                                atT[:, q * 512:(q + 1) * 512],
                                start=(jt == 0), stop=(jt == NT - 1))

                # ---- phase C: ELU on h'^T, transpose back, store ----
                with tc.tile_pool(name="elu", bufs=1) as pe:
                    for hh in range(2):
                        # ELU(x) = relu(x) + min(exp(x),1) - 1
                        trelu = pe.tile([64, N], F32, tag=f"trelu{hh}",
                                        name=f"trelu{hh}")
                        nc.scalar.activation(trelu[:], pso[hh][:], ACT.Relu)
                        texp0 = pe.tile([64, N], F32, tag=f"texp0{hh}",
                                        name=f"texp0{hh}")
                        nc.scalar.activation(texp0[:], pso[hh][:], ACT.Exp,
                                             scale=1.0)
                        texp = pe.tile([64, N], F32, tag=f"texp{hh}",
                                       name=f"texp{hh}")
                        nc.vector.tensor_scalar_min(texp[:], texp0[:], 1.0)
                        eluT = pe.tile([64, N], F16, tag=f"eluT{hh}",
                                       name=f"eluT{hh}")
                        nc.vector.scalar_tensor_tensor(
                            eluT[:], texp[:], -1.0, trelu[:],
                            AL.add, AL.add)
                        # [d=64, i] -> [i mod 128, it, d] via xbar transpose
                        nc.sync.dma_start_transpose(
                            out_sb[:].rearrange("p (t c) -> p t c", c=128)[
                                :, :, hh * D:(hh + 1) * D],
                            eluT[:])

            for hh in range(2):
                nc.sync.dma_start(
                    out_d[:, hh * D:(hh + 1) * D].rearrange(
                        "(t p) d -> p t d", p=128),
                    out_sb[:].rearrange("p (t c) -> p t c", c=128)[
                        :, :, hh * D:(hh + 1) * D])

    nc.compile()
    return nc


def _get_program():
    if "nc" not in _CACHE:
        _CACHE["nc"] = _build_program()
    return _CACHE["nc"]


def kernel(x, adj, type_onehot, Wq, Wk, W, a):
    from concourse.bass_utils import run_bass_kernel_spmd

    nc = _get_program()
    x = np.asarray(x, dtype=np.float32)
    adj = np.asarray(adj, dtype=np.int32)
    toh = np.asarray(type_onehot, dtype=np.float32)
    Wq = np.asarray(Wq, dtype=np.float32)
    Wk = np.asarray(Wk, dtype=np.float32)
    W = np.asarray(W, dtype=np.float32)
    a = np.asarray(a, dtype=np.float32)

    a1 = a[:, :D, 0]
    a2v = a[:, D:, 0]
    vq = np.einsum("hfd,hd->hf", Wq, a1)
    vk = np.einsum("hfd,hd->hf", Wk, a2v)

    adjT16 = [np.ascontiguousarray(adj[b].T.astype(np.float16))
              for b in range(B)]
    htT16 = []
    for b in range(B):
        m = np.zeros((FTP, N), dtype=np.float16)
        m[:F] = x[b].T
        m[F:FT] = toh[b].T
        htT16.append(m)

    in_maps = []
    for c in range(8):
        b, hp = c // 2, c % 2
        hs = slice(2 * hp, 2 * hp + 2)
        vqk = np.zeros((FTP, 4), dtype=np.float16)
        vqk[:FT, 0] = vq[2 * hp].astype(np.float16)
        vqk[:FT, 1] = vk[2 * hp].astype(np.float16)
        vqk[:FT, 2] = vq[2 * hp + 1].astype(np.float16)
        vqk[:FT, 3] = vk[2 * hp + 1].astype(np.float16)
        in_maps.append({
            "adjT": adjT16[b],
            "htT": htT16[b],
            "vqk": vqk,
            "w2": np.ascontiguousarray(W[hs].astype(np.float16)),
        })

    trace = bool(int(os.environ.get("GAT_TRACE", "0")))
    res = run_bass_kernel_spmd(nc, in_maps, core_ids=list(range(8)), trace=trace)
    _CACHE["last_result"] = res

    out = np.empty((B, N, H * D), dtype=np.float32)
    for c in range(8):
        b, hp = c // 2, c % 2
        out[b, :, 128 * hp:128 * (hp + 1)] = \
            res.results[c]["out"].astype(np.float32)
    return out
